# revision 7
# baseline (speedup 1.0000x reference)
"""Trainium2 Bass kernel for nn_CropModule: per-sample crop + bilinear resize.

Contract: kernel(img [128,3,480,480] f32, box [128,4] f32) -> [128, 150528] f32.

Strategy (pure data parallel, 16 samples per NeuronCore across 8 cores):
  * Host computes, per sample, the 240x240 crop window and two bilinear
    selection tables RyT [240,224] / RxT [240,224] (bilinear weight
    w[i,o] = relu(1 - |i - src(o)|), identical to the reference's
    scatter construction), packs window rows + tables into one bf16
    block [120, 2336] laid out exactly as the SBUF tile:
        free cols [0,1440)      window rows, (c, yc, x) order,
                                partition p = window row yc*120+p
        free cols [1440,1888)   RyT, (yc, oy) order
        free cols [1888,2336)   RxT, (xc, ox) order
    HBM buffers are partition-major ([120, nsamp*2336] / [112,
    nsamp*1344]) so batched DMAs get large per-partition descriptors.
  * Device, per sample and channel ("unit"), two accumulating bf16
    matmul passes (f32 PSUM accumulate):
        mid[x, oy] = sum_y W[y, x] * RyT[y, oy]      (pass V)
        out[oy, ox] = sum_x mid[x, oy] * RxT[x, ox]  (pass H)
    Units are software-pipelined: pass H of unit u-1 is emitted between
    pass V and pass H of unit u, so the tensor engine never stalls on
    the PSUM->SBUF cast (vector) between the passes. Input DMAs load 2
    samples, output DMAs store 4 samples.
  * Host unpacks to [B, 3, 224, 224] f32. bf16 end-to-end rel err vs the
    f32 reference is ~8e-3, well inside the 2e-2 gate.
"""
from contextlib import ExitStack

import ml_dtypes
import numpy as np

import concourse.bass as bass
import concourse.mybir as mybir
import concourse.tile as tile
from concourse.bass_utils import run_bass_kernel_spmd
from concourse.vector_clock import ScopedClock

IMG = 480
OUT = 224
WIN = 240
BATCH = 128
N_CORES = 8
NSAMP = BATCH // N_CORES

W_COLS = 3 * WIN * 2          # 1440: window, (c, yc, x)
T_COLS = 2 * OUT              # 448 per axis, (chunk, o)
IN_COLS = W_COLS + 2 * T_COLS  # 2336
OUT_COLS = 3 * 2 * OUT        # 1344: (c, oc, ox)
TY = W_COLS                   # RyT base col within a sample block
TX = W_COLS + T_COLS          # RxT base col

F32 = mybir.dt.float32
BF16 = mybir.dt.bfloat16
BF16_NP = ml_dtypes.bfloat16


# ---------------------------------------------------------------- host prep

def _prep(img, box):
    """-> inp [B, 120, 2336] bf16 (per-sample packed window+tables)."""
    B = box.shape[0]
    b = box.astype(np.float32) * np.float32(IMG)
    xa = np.trunc(b[:, 0] - np.float32(0.5) * b[:, 2]).astype(np.int32)
    ya = np.trunc(b[:, 1] - np.float32(0.5) * b[:, 3]).astype(np.int32)
    xb = np.trunc(b[:, 0] + np.float32(0.5) * b[:, 2]).astype(np.int32)
    yb = np.trunc(b[:, 1] + np.float32(0.5) * b[:, 3]).astype(np.int32)
    wy0 = np.minimum(ya, IMG - WIN)
    wx0 = np.minimum(xa, IMG - WIN)

    def axis_tab(ca, cb, w0):
        # src coord within the window for each output pixel, f32 as in ref
        cn = (cb - ca).astype(np.float32)[:, None]
        o = np.arange(OUT, dtype=np.float32)[None, :]
        s = np.clip((o + np.float32(0.5)) * cn / np.float32(OUT)
                    - np.float32(0.5), np.float32(0.0), cn - np.float32(1.0))
        s = s + (ca - w0).astype(np.float32)[:, None]          # [B, OUT]
        i = np.arange(WIN, dtype=np.float32)[None, :, None]
        tab = np.maximum(np.float32(0.0),
                         np.float32(1.0) - np.abs(i - s[:, None, :]))
        return tab                                              # [B, WIN, OUT]

    ty = axis_tab(ya, yb, wy0).astype(BF16_NP)
    tx = axis_tab(xa, xb, wx0).astype(BF16_NP)

    sidx = np.arange(B)[:, None, None, None]
    cidx = np.arange(3)[None, :, None, None]
    rows = (wy0[:, None] + np.arange(WIN))[:, None, :, None]
    cols = (wx0[:, None] + np.arange(WIN))[:, None, None, :]
    win = img[sidx, cidx, rows, cols].astype(BF16_NP)           # [B,3,240,240]

    inp = np.empty((B, 120, IN_COLS), dtype=BF16_NP)
    inp[:, :, :W_COLS] = (
        win.reshape(B, 3, 2, 120, WIN).transpose(0, 3, 1, 2, 4)
        .reshape(B, 120, W_COLS))
    inp[:, :, W_COLS:W_COLS + T_COLS] = (
        ty.reshape(B, 2, 120, OUT).transpose(0, 2, 1, 3).reshape(B, 120, T_COLS))
    inp[:, :, W_COLS + T_COLS:] = (
        tx.reshape(B, 2, 120, OUT).transpose(0, 2, 1, 3).reshape(B, 120, T_COLS))
    return inp


def _unpack(out_packed):
    """[B, 112, 1344] bf16 -> [B, 3*224*224] f32."""
    B = out_packed.shape[0]
    o = out_packed.reshape(B, 112, 3, 2, OUT).transpose(0, 2, 3, 1, 4)
    return np.ascontiguousarray(o).reshape(B, -1).astype(np.float32)


# ------------------------------------------------- walrus wait-limit fixups

class _SplitDrainTileContext(tile.TileContext):
    """The walrus build here rejects instructions carrying several sync
    waits; re-emit the kernel-tail drain's waits as single-wait NoOps,
    spread round-robin across engines so they retire in parallel."""

    def _drain_and_barrier(self, tick_clock, wait_clock):
        nc = self.nc
        probe = nc.sync.nop(nofuse=True, hint="drain_wait_probe")
        wait_clock.add_sem_waits(
            probe.ins, ScopedClock({None: tick_clock.global_clock}))
        si = probe.ins.sync_info
        waits = list(si.on_wait) if si is not None else []
        if si is not None:
            si.on_wait = waits[:1]
        wait_engines = [nc.sync, nc.scalar, nc.vector, nc.tensor]
        for i, w in enumerate(waits[1:]):
            eng = wait_engines[i % len(wait_engines)]
            n = eng.nop(nofuse=True, hint="drain_wait_split")
            n.ins.sync_info = mybir.SyncInfo(on_wait=[w], on_update=[])
        nc.sync.drain()

        nc.all_engine_barrier()
        assert self.sems is not None
        popped = nc._tile_sem_poison_stack.pop()
        assert popped is self._sem_poison
        nc.clear_and_free_semaphores(list(self.sems.allocated().values()))
        nc.all_engine_barrier()


def _split_sync_waits(nc, max_waits=1):
    ctr = 0
    for fn in nc.m.functions:
        for blk in fn.blocks:
            out = []
            for inst in blk.instructions:
                si = getattr(inst, "sync_info", None)
                waits = list(si.on_wait) if si is not None and si.on_wait else []
                if len(waits) > max_waits:
                    for w in waits[:-max_waits]:
                        ctr += 1
                        out.append(mybir.InstNoOp(
                            name=f"wsplit_{ctr}",
                            engine=inst.engine,
                            ins=[], outs=[],
                            sync_info=mybir.SyncInfo(on_wait=[w], on_update=[])))
                    si.on_wait = waits[-max_waits:]
                out.append(inst)
            blk.instructions = out


# ------------------------------------------------------------ device kernel

IN_BATCH = 2    # samples per input DMA
OUT_BATCH = 2   # samples per output DMA


def build_kernel(nsamp=NSAMP, n_cores=N_CORES):
    nc = bass.Bass("TRN2", target_bir_lowering=False, debug=False,
                   num_devices=n_cores)
    inp = nc.dram_tensor("inp", [120, nsamp * IN_COLS], BF16,
                         kind="ExternalInput")
    out = nc.dram_tensor("out", [112, nsamp * OUT_COLS], BF16,
                         kind="ExternalOutput")

    with _SplitDrainTileContext(nc) as tc, ExitStack() as ctx:
        inpp = ctx.enter_context(tc.tile_pool(name="inpp", bufs=4))
        midp = ctx.enter_context(tc.tile_pool(name="midp", bufs=4))
        outp = ctx.enter_context(tc.tile_pool(name="outp", bufs=2))
        midps = ctx.enter_context(tc.tile_pool(name="midps", bufs=4, space="PSUM"))
        outps = ctx.enter_context(tc.tile_pool(name="outps", bufs=3, space="PSUM"))

        pending = None  # (sb, sbase, mid_sb, out_sb, ocol, is_group_last)

        def emit_h(u):
            sb, sbase, mid_sb, out_sb, ocol, last = u
            out_ps = outps.tile([112, 2 * OUT], F32)
            for oc in range(2):
                for xc in range(2):
                    nc.tensor.matmul(
                        out_ps[:, oc * OUT:(oc + 1) * OUT],
                        lhsT=mid_sb[:, xc * OUT + oc * 112:
                                    xc * OUT + oc * 112 + 112],
                        rhs=sb[:, sbase + TX + xc * OUT:
                               sbase + TX + (xc + 1) * OUT],
                        start=(xc == 0), stop=(xc == 1))
            nc.scalar.copy(out=out_sb[:, ocol:ocol + 2 * OUT], in_=out_ps[:])
            if last:
                g0 = last[0]
                nc.scalar.dma_start(
                    out.ap()[:, g0 * OUT_COLS:(g0 + OUT_BATCH) * OUT_COLS],
                    out_sb[:])

        sb = out_sb = None
        for s in range(nsamp):
            if s % IN_BATCH == 0:
                sb = inpp.tile([120, IN_BATCH * IN_COLS], BF16)
                if s == 0:
                    # split the first load so compute starts one sample sooner
                    for j in range(IN_BATCH):
                        nc.sync.dma_start(
                            sb[:, j * IN_COLS:(j + 1) * IN_COLS],
                            inp.ap()[:, j * IN_COLS:(j + 1) * IN_COLS])
                else:
                    nc.sync.dma_start(
                        sb[:], inp.ap()[:, s * IN_COLS:(s + IN_BATCH) * IN_COLS])
            sbase = (s % IN_BATCH) * IN_COLS
            if s % OUT_BATCH == 0:
                out_sb = outp.tile([112, OUT_BATCH * OUT_COLS], BF16)
            for c in range(3):
                mid_ps = midps.tile([120, 2 * OUT], F32)
                for xc in range(2):
                    for yc in range(2):
                        w0 = sbase + c * 480 + yc * WIN + xc * 120
                        nc.tensor.matmul(
                            mid_ps[:, xc * OUT:(xc + 1) * OUT],
                            lhsT=sb[:, w0:w0 + 120],
                            rhs=sb[:, sbase + TY + yc * OUT:
                                   sbase + TY + (yc + 1) * OUT],
                            start=(yc == 0), stop=(yc == 1))
                mid_sb = midp.tile([120, 2 * OUT], BF16)
                nc.vector.tensor_copy(mid_sb[:], mid_ps[:])
                if pending is not None:
                    emit_h(pending)
                last = ((s - OUT_BATCH + 1,)
                        if (s % OUT_BATCH == OUT_BATCH - 1 and c == 2) else None)
                ocol = (s % OUT_BATCH) * OUT_COLS + c * 2 * OUT
                pending = (sb, sbase, mid_sb, out_sb, ocol, last)
        emit_h(pending)
    _split_sync_waits(nc)
    return nc


_NC_CACHE = {}


def _run(img, box, trace=False, trace_kwargs=None):
    key = (NSAMP, N_CORES)
    if key not in _NC_CACHE:
        _NC_CACHE[key] = build_kernel(*key)
    nc = _NC_CACHE[key]
    inp = _prep(np.asarray(img, dtype=np.float32),
                np.asarray(box, dtype=np.float32))
    in_maps = []
    for cid in range(N_CORES):
        lo = cid * NSAMP
        blk = inp[lo:lo + NSAMP].transpose(1, 0, 2).reshape(120, -1)
        in_maps.append({"inp": np.ascontiguousarray(blk)})
    res = run_bass_kernel_spmd(nc, in_maps, list(range(N_CORES)), trace=trace,
                               **(trace_kwargs or {}))
    parts = []
    for i in range(N_CORES):
        o = res.results[i]["out"].reshape(112, NSAMP, OUT_COLS)
        parts.append(o.transpose(1, 0, 2))
    return _unpack(np.concatenate(parts, axis=0)), res


def kernel(img, box):
    out, _ = _run(img, box, trace=False)
    return out


# revision 8
# speedup vs baseline: 1.0031x; 1.0031x over previous
"""Trainium2 Bass kernel for nn_CropModule: per-sample crop + bilinear resize.

Contract: kernel(img [128,3,480,480] f32, box [128,4] f32) -> [128, 150528] f32.

Strategy (pure data parallel, 16 samples per NeuronCore across 8 cores):
  * Host computes, per sample, the 240x240 crop window and two bilinear
    selection tables RyT [240,224] / RxT [240,224] (bilinear weight
    w[i,o] = relu(1 - |i - src(o)|), identical to the reference's
    scatter construction), packs window rows + tables into one bf16
    block [120, 2336] laid out exactly as the SBUF tile:
        free cols [0,1440)      window rows, (c, yc, x) order,
                                partition p = window row yc*120+p
        free cols [1440,1888)   RyT, (yc, oy) order
        free cols [1888,2336)   RxT, (xc, ox) order
    HBM buffers are partition-major ([120, nsamp*2336] / [112,
    nsamp*1344]) so batched DMAs get large per-partition descriptors.
  * Device, per sample and channel ("unit"), two accumulating bf16
    matmul passes (f32 PSUM accumulate):
        mid[x, oy] = sum_y W[y, x] * RyT[y, oy]      (pass V)
        out[oy, ox] = sum_x mid[x, oy] * RxT[x, ox]  (pass H)
    Units are software-pipelined: pass H of unit u-1 is emitted between
    pass V and pass H of unit u, so the tensor engine never stalls on
    the PSUM->SBUF cast (vector) between the passes. Input DMAs load 2
    samples, output DMAs store 4 samples.
  * Host unpacks to [B, 3, 224, 224] f32. bf16 end-to-end rel err vs the
    f32 reference is ~8e-3, well inside the 2e-2 gate.
"""
from contextlib import ExitStack

import ml_dtypes
import numpy as np

import concourse.bass as bass
import concourse.mybir as mybir
import concourse.tile as tile
from concourse.bass_utils import run_bass_kernel_spmd
from concourse.vector_clock import ScopedClock

IMG = 480
OUT = 224
WIN = 240
BATCH = 128
N_CORES = 8
NSAMP = BATCH // N_CORES

W_COLS = 3 * WIN * 2          # 1440: window, (c, yc, x)
T_COLS = 2 * OUT              # 448 per axis, (chunk, o)
IN_COLS = W_COLS + 2 * T_COLS  # 2336
OUT_COLS = 3 * 2 * OUT        # 1344: (c, oc, ox)
TY = W_COLS                   # RyT base col within a sample block
TX = W_COLS + T_COLS          # RxT base col

F32 = mybir.dt.float32
BF16 = mybir.dt.bfloat16
BF16_NP = ml_dtypes.bfloat16


# ---------------------------------------------------------------- host prep

def _prep(img, box):
    """-> inp [B, 120, 2336] bf16 (per-sample packed window+tables)."""
    B = box.shape[0]
    b = box.astype(np.float32) * np.float32(IMG)
    xa = np.trunc(b[:, 0] - np.float32(0.5) * b[:, 2]).astype(np.int32)
    ya = np.trunc(b[:, 1] - np.float32(0.5) * b[:, 3]).astype(np.int32)
    xb = np.trunc(b[:, 0] + np.float32(0.5) * b[:, 2]).astype(np.int32)
    yb = np.trunc(b[:, 1] + np.float32(0.5) * b[:, 3]).astype(np.int32)
    wy0 = np.minimum(ya, IMG - WIN)
    wx0 = np.minimum(xa, IMG - WIN)

    def axis_tab(ca, cb, w0):
        # src coord within the window for each output pixel, f32 as in ref
        cn = (cb - ca).astype(np.float32)[:, None]
        o = np.arange(OUT, dtype=np.float32)[None, :]
        s = np.clip((o + np.float32(0.5)) * cn / np.float32(OUT)
                    - np.float32(0.5), np.float32(0.0), cn - np.float32(1.0))
        s = s + (ca - w0).astype(np.float32)[:, None]          # [B, OUT]
        i = np.arange(WIN, dtype=np.float32)[None, :, None]
        tab = np.maximum(np.float32(0.0),
                         np.float32(1.0) - np.abs(i - s[:, None, :]))
        return tab                                              # [B, WIN, OUT]

    ty = axis_tab(ya, yb, wy0).astype(BF16_NP)
    tx = axis_tab(xa, xb, wx0).astype(BF16_NP)

    sidx = np.arange(B)[:, None, None, None]
    cidx = np.arange(3)[None, :, None, None]
    rows = (wy0[:, None] + np.arange(WIN))[:, None, :, None]
    cols = (wx0[:, None] + np.arange(WIN))[:, None, None, :]
    win = img[sidx, cidx, rows, cols].astype(BF16_NP)           # [B,3,240,240]

    inp = np.empty((B, 120, IN_COLS), dtype=BF16_NP)
    inp[:, :, :W_COLS] = (
        win.reshape(B, 3, 2, 120, WIN).transpose(0, 3, 1, 2, 4)
        .reshape(B, 120, W_COLS))
    inp[:, :, W_COLS:W_COLS + T_COLS] = (
        ty.reshape(B, 2, 120, OUT).transpose(0, 2, 1, 3).reshape(B, 120, T_COLS))
    inp[:, :, W_COLS + T_COLS:] = (
        tx.reshape(B, 2, 120, OUT).transpose(0, 2, 1, 3).reshape(B, 120, T_COLS))
    return inp


def _unpack(out_packed):
    """[B, 112, 1344] bf16 -> [B, 3*224*224] f32."""
    B = out_packed.shape[0]
    o = out_packed.reshape(B, 112, 3, 2, OUT).transpose(0, 2, 3, 1, 4)
    return np.ascontiguousarray(o).reshape(B, -1).astype(np.float32)


# ------------------------------------------------- walrus wait-limit fixups

class _SplitDrainTileContext(tile.TileContext):
    """The walrus build here rejects instructions carrying several sync
    waits; re-emit the kernel-tail drain's waits as single-wait NoOps,
    spread round-robin across engines so they retire in parallel."""

    def _drain_and_barrier(self, tick_clock, wait_clock):
        nc = self.nc
        probe = nc.sync.nop(nofuse=True, hint="drain_wait_probe")
        wait_clock.add_sem_waits(
            probe.ins, ScopedClock({None: tick_clock.global_clock}))
        si = probe.ins.sync_info
        waits = list(si.on_wait) if si is not None else []
        if si is not None:
            si.on_wait = waits[:1]
        wait_engines = [nc.sync, nc.scalar, nc.vector, nc.tensor]
        for i, w in enumerate(waits[1:]):
            eng = wait_engines[i % len(wait_engines)]
            n = eng.nop(nofuse=True, hint="drain_wait_split")
            n.ins.sync_info = mybir.SyncInfo(on_wait=[w], on_update=[])
        nc.sync.drain()

        nc.all_engine_barrier()
        assert self.sems is not None
        popped = nc._tile_sem_poison_stack.pop()
        assert popped is self._sem_poison
        nc.clear_and_free_semaphores(list(self.sems.allocated().values()))
        nc.all_engine_barrier()


def _split_sync_waits(nc, max_waits=1):
    ctr = 0
    for fn in nc.m.functions:
        for blk in fn.blocks:
            out = []
            for inst in blk.instructions:
                si = getattr(inst, "sync_info", None)
                waits = list(si.on_wait) if si is not None and si.on_wait else []
                if len(waits) > max_waits:
                    for w in waits[:-max_waits]:
                        ctr += 1
                        out.append(mybir.InstNoOp(
                            name=f"wsplit_{ctr}",
                            engine=inst.engine,
                            ins=[], outs=[],
                            sync_info=mybir.SyncInfo(on_wait=[w], on_update=[])))
                    si.on_wait = waits[-max_waits:]
                out.append(inst)
            blk.instructions = out


# ------------------------------------------------------------ device kernel

IN_BATCH = 2    # samples per input DMA
OUT_BATCH = 2   # samples per output DMA


def build_kernel(nsamp=NSAMP, n_cores=N_CORES):
    nc = bass.Bass("TRN2", target_bir_lowering=False, debug=False,
                   num_devices=n_cores)
    inp = nc.dram_tensor("inp", [120, nsamp * IN_COLS], BF16,
                         kind="ExternalInput")
    out = nc.dram_tensor("out", [112, nsamp * OUT_COLS], BF16,
                         kind="ExternalOutput")

    with _SplitDrainTileContext(nc) as tc, ExitStack() as ctx:
        inpp = ctx.enter_context(tc.tile_pool(name="inpp", bufs=4))
        midp = ctx.enter_context(tc.tile_pool(name="midp", bufs=4))
        outp = ctx.enter_context(tc.tile_pool(name="outp", bufs=2))
        midps = ctx.enter_context(tc.tile_pool(name="midps", bufs=4, space="PSUM"))
        outps = ctx.enter_context(tc.tile_pool(name="outps", bufs=3, space="PSUM"))

        pending = None  # (sb, sbase, mid_sb, out_sb, ocol, is_group_last)

        def emit_h(u):
            sb, sbase, mid_sb, out_sb, ocol, last = u
            out_ps = outps.tile([112, 2 * OUT], F32)
            for oc in range(2):
                for xc in range(2):
                    nc.tensor.matmul(
                        out_ps[:, oc * OUT:(oc + 1) * OUT],
                        lhsT=mid_sb[:, xc * OUT + oc * 112:
                                    xc * OUT + oc * 112 + 112],
                        rhs=sb[:, sbase + TX + xc * OUT:
                               sbase + TX + (xc + 1) * OUT],
                        start=(xc == 0), stop=(xc == 1))
            nc.scalar.copy(out=out_sb[:, ocol:ocol + 2 * OUT], in_=out_ps[:])
            if last:
                g0 = last[0]
                # keep both HWDGE queues carrying a balanced in+out mix
                eng = nc.scalar if (g0 // OUT_BATCH) % 2 == 0 else nc.sync
                eng.dma_start(
                    out.ap()[:, g0 * OUT_COLS:(g0 + OUT_BATCH) * OUT_COLS],
                    out_sb[:])

        sb = out_sb = None
        for s in range(nsamp):
            if s % IN_BATCH == 0:
                sb = inpp.tile([120, IN_BATCH * IN_COLS], BF16)
                eng = nc.sync if (s // IN_BATCH) % 2 == 0 else nc.scalar
                if s == 0:
                    # split the first load so compute starts one sample sooner
                    for j in range(IN_BATCH):
                        eng.dma_start(
                            sb[:, j * IN_COLS:(j + 1) * IN_COLS],
                            inp.ap()[:, j * IN_COLS:(j + 1) * IN_COLS])
                else:
                    eng.dma_start(
                        sb[:], inp.ap()[:, s * IN_COLS:(s + IN_BATCH) * IN_COLS])
            sbase = (s % IN_BATCH) * IN_COLS
            if s % OUT_BATCH == 0:
                out_sb = outp.tile([112, OUT_BATCH * OUT_COLS], BF16)
            for c in range(3):
                mid_ps = midps.tile([120, 2 * OUT], F32)
                for xc in range(2):
                    for yc in range(2):
                        w0 = sbase + c * 480 + yc * WIN + xc * 120
                        nc.tensor.matmul(
                            mid_ps[:, xc * OUT:(xc + 1) * OUT],
                            lhsT=sb[:, w0:w0 + 120],
                            rhs=sb[:, sbase + TY + yc * OUT:
                                   sbase + TY + (yc + 1) * OUT],
                            start=(yc == 0), stop=(yc == 1))
                mid_sb = midp.tile([120, 2 * OUT], BF16)
                nc.vector.tensor_copy(mid_sb[:], mid_ps[:])
                if pending is not None:
                    emit_h(pending)
                last = ((s - OUT_BATCH + 1,)
                        if (s % OUT_BATCH == OUT_BATCH - 1 and c == 2) else None)
                ocol = (s % OUT_BATCH) * OUT_COLS + c * 2 * OUT
                pending = (sb, sbase, mid_sb, out_sb, ocol, last)
        emit_h(pending)
    _split_sync_waits(nc)
    return nc


_NC_CACHE = {}


def _run(img, box, trace=False, trace_kwargs=None):
    key = (NSAMP, N_CORES)
    if key not in _NC_CACHE:
        _NC_CACHE[key] = build_kernel(*key)
    nc = _NC_CACHE[key]
    inp = _prep(np.asarray(img, dtype=np.float32),
                np.asarray(box, dtype=np.float32))
    in_maps = []
    for cid in range(N_CORES):
        lo = cid * NSAMP
        blk = inp[lo:lo + NSAMP].transpose(1, 0, 2).reshape(120, -1)
        in_maps.append({"inp": np.ascontiguousarray(blk)})
    res = run_bass_kernel_spmd(nc, in_maps, list(range(N_CORES)), trace=trace,
                               **(trace_kwargs or {}))
    parts = []
    for i in range(N_CORES):
        o = res.results[i]["out"].reshape(112, NSAMP, OUT_COLS)
        parts.append(o.transpose(1, 0, 2))
    return _unpack(np.concatenate(parts, axis=0)), res


def kernel(img, box):
    out, _ = _run(img, box, trace=False)
    return out
